# revision 1
# baseline (speedup 1.0000x reference)
"""NMI loss (normalized mutual information over soft histograms) on 8 trn2 cores.

Voxel-sharded (per sharding hint): each core processes N/8 = 262144 voxels.
Per 128-voxel group it builds dense I_a / I_b rows (32 Gaussian-window bins)
with DVE (subtract) + ACT (square, exp), normalizes I_a by its row sum, and
accumulates ONE 33x33 Gram matrix on the TensorEngine:
    lhsT = [I_an | 1/S_b]  (bf16), rhs = [I_b | 1]  (bf16)
giving   out[0:32,0:32] = sum I_an*I_b   (N*pab partial)
         out[0:32,32]   = sum I_an       (N*pa  partial)
         out[32,0:32]   = sum I_b/S_b    (N*pb  partial)
The 8 partial 33x33 stats go to the host, which sums them and does the tiny
log-MI reduction (1024 elements) exactly as the reference.

Raw Bass blocks (manual semaphores): the Tile layer's multi-wait sync_info is
rejected by this container's walrus ("Too many sync wait commands"), so the
pipeline below uses standalone wait_ge instructions and depth-2 buffering.
"""

import sys
import numpy as np

sys.path.insert(0, "/opt/trn_rl_repo")

NCORES = 8
P = 128
B = 32                     # bins
S = B + 1                  # slot width (bins + 1 extra column)
NVOX_TOTAL = 128 ** 3      # 2097152
NVOX = NVOX_TOTAL // NCORES
COLS = NVOX // P           # 2048 voxel-columns per core
CHUNK = 64                 # voxel-columns per chunk
NCHUNK = COLS // CHUNK     # 32

# replicate reference's f32 constant computation
_BC = np.linspace(0.0, 1.0, B, dtype=np.float32)
_SIGMA = (np.mean(np.diff(_BC)) * np.float32(0.5)).astype(np.float32)
_PRETERM = (np.float32(1.0) / (np.float32(2.0) * _SIGMA * _SIGMA)).astype(np.float32)

_CACHE = {}


def _build_nc(reps=1):
    from contextlib import ExitStack
    from concourse import bass, mybir

    f32 = mybir.dt.float32
    bf16 = mybir.dt.bfloat16
    AX = mybir.AxisListType
    AF = mybir.ActivationFunctionType

    nc = bass.Bass()
    a_d = nc.dram_tensor("a", [NCHUNK, P, CHUNK], f32, kind="ExternalInput")
    b_d = nc.dram_tensor("b", [NCHUNK, P, CHUNK], f32, kind="ExternalInput")
    iota_d = nc.dram_tensor("iota", [P, B], f32, kind="ExternalInput")
    out_d = nc.dram_tensor("stats", [S, S], f32, kind="ExternalOutput")

    FB = CHUNK * B   # 2048
    FS = CHUNK * S   # 2112

    with ExitStack() as ctx:
        e = ctx.enter_context
        iota_sb = e(nc.sbuf_tensor("iota_sb", [P, B], f32))
        a_t = [e(nc.sbuf_tensor(f"a_t{i}", [P, CHUNK], f32)) for i in range(2)]
        b_t = [e(nc.sbuf_tensor(f"b_t{i}", [P, CHUNK], f32)) for i in range(2)]
        d_a = [e(nc.sbuf_tensor(f"d_a{i}", [P, FB], f32)) for i in range(2)]
        d_b = [e(nc.sbuf_tensor(f"d_b{i}", [P, FB], f32)) for i in range(2)]
        sq_a = [e(nc.sbuf_tensor(f"sq_a{i}", [P, FB], f32)) for i in range(2)]
        sq_b = [e(nc.sbuf_tensor(f"sq_b{i}", [P, FB], f32)) for i in range(2)]
        ia = [e(nc.sbuf_tensor(f"ia{i}", [P, FB], f32)) for i in range(2)]
        ach = [e(nc.sbuf_tensor(f"ach{i}", [P, FS], bf16)) for i in range(2)]
        bch = [e(nc.sbuf_tensor(f"bch{i}", [P, FS], bf16)) for i in range(2)]
        sa = e(nc.sbuf_tensor("sa_sb", [P, CHUNK], f32))
        isa = e(nc.sbuf_tensor("isa_sb", [P, CHUNK], f32))
        sb = e(nc.sbuf_tensor("sb_sb", [P, CHUNK], f32))
        isb = e(nc.sbuf_tensor("isb_sb", [P, CHUNK], f32))
        stats_sb = e(nc.sbuf_tensor("stats_sb", [S, S], f32))
        acc = e(nc.psum_tensor("acc_ps", [S, S], f32))

        s_iota = e(nc.semaphore("s_iota"))
        s_dma_a = e(nc.semaphore("s_dma_a"))
        s_dma_b = e(nc.semaphore("s_dma_b"))
        s_suba = e(nc.semaphore("s_suba"))
        s_subb = e(nc.semaphore("s_subb"))
        s_expa = e(nc.semaphore("s_expa"))
        s_expb = e(nc.semaphore("s_expb"))
        s_ach = e(nc.semaphore("s_ach"))
        s_ones = e(nc.semaphore("s_ones"))
        s_pe = e(nc.semaphore("s_pe"))
        s_done = e(nc.semaphore("s_done"))
        s_out = e(nc.semaphore("s_out"))
        block = e(nc.Block())

        def r3(ap, inner):
            return ap[:, :].rearrange("p (v i) -> p v i", i=inner)

        iota_bc = (
            iota_sb[:, :]
            .rearrange("p (o i) -> p o i", o=1)
            .broadcast_to([P, CHUNK, B])
        )

        G = reps * NCHUNK

        @block.sync
        def _(sync):
            sync.dma_start(iota_sb[:, :], iota_d[:, :]).then_inc(s_iota, 16)
            for g in range(G):
                c = g % NCHUNK
                if g >= 2:
                    sync.wait_ge(s_suba, g - 1)
                sync.dma_start(a_t[g % 2][:, :], a_d[c]).then_inc(s_dma_a, 16)
                if g >= 2:
                    sync.wait_ge(s_subb, g - 1)
                sync.dma_start(b_t[g % 2][:, :], b_d[c]).then_inc(s_dma_b, 16)

        @block.gpsimd
        def _(gpsimd):
            # ones in the B-side extra slot, once per buffer
            for k in range(2):
                gpsimd.memset(r3(bch[k], S)[:, :, B : B + 1], 1.0).then_inc(s_ones, 1)
            gpsimd.wait_ge(s_done, 1)
            gpsimd.dma_start(out_d[:, :], stats_sb[:, :]).then_inc(s_out, 16)
            gpsimd.wait_ge(s_out, 16)

        @block.vector
        def _(vector):
            vector.wait_ge(s_iota, 16)
            for c in range(G):
                k = c % 2
                vector.wait_ge(s_dma_a, 16 * (c + 1))
                vector.tensor_sub(
                    r3(d_a[k], B),
                    a_t[k][:, :].broadcast_to([P, CHUNK, B]),
                    iota_bc,
                ).then_inc(s_suba, 1)
                vector.wait_ge(s_dma_b, 16 * (c + 1))
                vector.tensor_sub(
                    r3(d_b[k], B),
                    b_t[k][:, :].broadcast_to([P, CHUNK, B]),
                    iota_bc,
                ).then_inc(s_subb, 1)

                vector.wait_ge(s_expa, c + 1)
                vector.reduce_sum(sa[:, :], r3(ia[k], B), axis=AX.X)
                vector.reciprocal(isa[:, :], sa[:, :])
                vector.wait_ge(s_expb, c + 1)
                vector.reduce_sum(sb[:, :], r3(bch[k], S)[:, :, 0:B], axis=AX.X)
                vector.reciprocal(isb[:, :], sb[:, :])

                if c >= 2:
                    vector.wait_ge(s_pe, c - 1)
                vector.tensor_mul(
                    r3(ach[k], S)[:, :, 0:B],
                    r3(ia[k], B),
                    isa[:, :]
                    .rearrange("p (v o) -> p v o", o=1)
                    .broadcast_to([P, CHUNK, B]),
                )
                vector.tensor_copy(
                    r3(ach[k], S)[:, :, B : B + 1],
                    isb[:, :].rearrange("p (v o) -> p v o", o=1),
                ).then_inc(s_ach, 1)

            vector.wait_ge(s_pe, G)
            vector.tensor_copy(stats_sb[:, :], acc[:, :]).then_inc(s_done, 1)

        @block.scalar
        def _(scalar):
            for c in range(G):
                k = c % 2
                scalar.wait_ge(s_suba, c + 1)
                scalar.activation(sq_a[k][:, :], d_a[k][:, :], AF.Square)
                if c >= 2:
                    scalar.wait_ge(s_ach, c - 1)
                scalar.activation(
                    ia[k][:, :], sq_a[k][:, :], AF.Exp, scale=float(-_PRETERM)
                ).then_inc(s_expa, 1)

                scalar.wait_ge(s_subb, c + 1)
                scalar.activation(sq_b[k][:, :], d_b[k][:, :], AF.Square)
                if c >= 2:
                    scalar.wait_ge(s_pe, c - 1)
                elif c == 0:
                    scalar.wait_ge(s_ones, 2)
                scalar.activation(
                    r3(bch[k], S)[:, :, 0:B],
                    r3(sq_b[k], B),
                    AF.Exp,
                    scale=float(-_PRETERM),
                ).then_inc(s_expb, 1)

        @block.tensor
        def _(tensor):
            for c in range(G):
                k = c % 2
                tensor.wait_ge(s_ach, c + 1)
                tensor.wait_ge(s_expb, c + 1)
                for v in range(CHUNK):
                    first = c % NCHUNK == 0 and v == 0
                    last = c % NCHUNK == NCHUNK - 1 and v == CHUNK - 1
                    mm = tensor.matmul(
                        acc[:, :],
                        ach[k][:, v * S : (v + 1) * S],
                        bch[k][:, v * S : (v + 1) * S],
                        start=first,
                        stop=last,
                    )
                    if v == CHUNK - 1:
                        mm.then_inc(s_pe, 1)

    return nc


def _get_nc():
    if "nc" not in _CACHE:
        _CACHE["nc"] = _build_nc()
    return _CACHE["nc"]


def run_device(a_flat, b_flat, trace=False):
    """Run the per-core bass kernel on 8 cores; returns (stats_sum, bass_results)."""
    from concourse.bass_utils import run_bass_kernel_spmd

    nc = _get_nc()
    iota_tile = np.tile(_BC[None, :], (P, 1)).astype(np.float32)

    def shard(x, i):
        sl = x[i * NVOX : (i + 1) * NVOX].reshape(P, NCHUNK, CHUNK)
        return np.ascontiguousarray(sl.transpose(1, 0, 2))

    in_maps = []
    for i in range(NCORES):
        in_maps.append(
            {"a": shard(a_flat, i), "b": shard(b_flat, i), "iota": iota_tile}
        )
    kw = {}
    if trace:
        kw.update(trace=True, trace_cores=[0])
    res = run_bass_kernel_spmd(nc, in_maps, list(range(NCORES)), **kw)
    stats = np.zeros((S, S), np.float64)
    for r in res.results:
        stats += np.asarray(r["stats"], np.float64)
    return stats, res


def finish(stats):
    n = float(NVOX_TOTAL)
    pab = stats[0:B, 0:B] / n
    pa = stats[0:B, B] / n
    pb = stats[B, 0:B] / n
    eps = 1.4e-45
    papb = np.outer(pa, pb) + eps
    mi = np.sum(pab * np.log(pab / papb + eps))
    return np.array([-mi], dtype=np.float32)


def kernel(actual, target):
    a = np.clip(np.asarray(actual, np.float32).reshape(-1), 0.0, 1.0)
    b = np.clip(np.asarray(target, np.float32).reshape(-1), 0.0, 1.0)
    stats, _ = run_device(a, b)
    return finish(stats)



# revision 4
# speedup vs baseline: 1.2426x; 1.2426x over previous
"""NMI loss (normalized mutual information over soft histograms) on 8 trn2
NeuronCores.

Voxel-sharded: each core processes N/8 = 262144 voxels as 128 partitions x
2048 voxel-columns in 4 chunks of V=512. Per chunk (bin-major bf16 layout
[P, (bin, vox)]):
- d' = sqrt(pre)*a - sqrt(pre)*bc_j via 32 per-bin two-scalar
  tensor_scalar ops (DVE 4x perf mode, ~200ns each; no iota tensor).
- a-side square: one in-place tensor_tensor self-mult (DVE 2x mode);
  b-side square runs on the ACT engine (Square activation) for balance.
- exp via ACT with scale=-1 (in place).
- row sums: L1 pairwise add on gpsimd/Pool; L2..L5 in-place TT adds on
  DVE (tensor_reduce has no DVE fast mode, a pairwise tree is ~2x).
- 1/S via a minimax quadratic (3 small f32 ops) instead of the 6
  cycle-per-element reciprocal.
- normalization mul in two halves; the TensorEngine starts each chunk's
  first V/2 strided 33-col matmuls after the first half, accumulating
  one 33x33 Gram matrix: lhsT = [I_an | 1/S_b], rhs = [I_b | 1], giving
  N*pab / N*pa / N*pb. Host sums the 8 partial stats and does the tiny
  log-MI reduction.
Engines are software-pipelined with double buffering and manual
semaphores (the Tile layer's multi-wait sync_info is rejected by this
container's walrus).
"""

import sys
import numpy as np

sys.path.insert(0, "/opt/trn_rl_repo")

NCORES = 8
P = 128
B = 32
S = B + 1
NVOX_TOTAL = 128 ** 3
NVOX = NVOX_TOTAL // NCORES
COLS = NVOX // P           # 2048
VMAX = 512
VS = [512, 512, 512, 512]   # sums to 2048
assert sum(VS) == COLS
NCH = len(VS)
FSMAX = S * VMAX

_BC = np.linspace(0.0, 1.0, B, dtype=np.float32)
_SIGMA = (np.mean(np.diff(_BC)) * np.float32(0.5)).astype(np.float32)
_PRETERM = (np.float32(1.0) / (np.float32(2.0) * _SIGMA * _SIGMA)).astype(np.float32)
_SQP = np.sqrt(_PRETERM).astype(np.float32)

_RC2 = np.float32(-0.38195263)
_RC1 = np.float32(0.2281786)
_RC0 = np.float32(1.11185786)

_CACHE = {}


def _build_nc():
    from contextlib import ExitStack
    from concourse import bass, mybir

    f32 = mybir.dt.float32
    bf16 = mybir.dt.bfloat16
    AF = mybir.ActivationFunctionType
    ALU = mybir.AluOpType

    nc = bass.Bass()
    a_d = nc.dram_tensor("a", [P, COLS], f32, kind="ExternalInput")
    b_d = nc.dram_tensor("b", [P, COLS], f32, kind="ExternalInput")
    out_d = nc.dram_tensor("stats", [S, S], f32, kind="ExternalOutput")

    OFF = np.cumsum([0] + VS)[:-1]

    with ExitStack() as ctx:
        e = ctx.enter_context
        tfa = [e(nc.sbuf_tensor(f"tfa{k}", [P, VMAX], f32)) for k in range(2)]
        tfb = [e(nc.sbuf_tensor(f"tfb{k}", [P, VMAX], f32)) for k in range(2)]
        tba = [e(nc.sbuf_tensor(f"tba{k}", [P, VMAX], bf16)) for k in range(2)]
        tbb = [e(nc.sbuf_tensor(f"tbb{k}", [P, VMAX], bf16)) for k in range(2)]
        ach = [e(nc.sbuf_tensor(f"ach{k}", [P, FSMAX], bf16)) for k in range(2)]
        bch = [e(nc.sbuf_tensor(f"bch{k}", [P, FSMAX], bf16)) for k in range(2)]
        t16a = e(nc.sbuf_tensor("t16a", [P, 16 * VMAX], bf16))
        t16b = e(nc.sbuf_tensor("t16b", [P, 16 * VMAX], bf16))
        s1 = e(nc.sbuf_tensor("s1", [P, VMAX], f32))
        h1 = e(nc.sbuf_tensor("h1", [P, VMAX], f32))
        y1 = e(nc.sbuf_tensor("y1", [P, VMAX], f32))
        yb = e(nc.sbuf_tensor("yb", [P, VMAX], bf16))
        stats_sb = e(nc.sbuf_tensor("stats_sb", [S, S], f32))
        acc = e(nc.psum_tensor("acc_ps", [S, S], f32))

        s_dma_a = e(nc.semaphore("s_dma_a"))
        s_dma_b = e(nc.semaphore("s_dma_b"))
        s_sub_b = e(nc.semaphore("s_sub_b"))
        s_sq_a = e(nc.semaphore("s_sq_a"))
        s_exp_a = e(nc.semaphore("s_exp_a"))
        s_exp_b = e(nc.semaphore("s_exp_b"))
        s_l1a = e(nc.semaphore("s_l1a"))
        s_l1b = e(nc.semaphore("s_l1b"))
        s_isb = e(nc.semaphore("s_isb"))
        s_mul1 = e(nc.semaphore("s_mul1"))
        s_mul2 = e(nc.semaphore("s_mul2"))
        s_pe = e(nc.semaphore("s_pe"))
        s_done = e(nc.semaphore("s_done"))
        s_out = e(nc.semaphore("s_out"))
        block = e(nc.Block())

        @block.sync
        def _(sync):
            for c in range(NCH):
                k = c % 2
                V = VS[c]
                off = int(OFF[c])
                if c >= 2:
                    sync.wait_ge(s_sub_b, c - 1)
                sync.dma_start(
                    tfb[k][:, 0:V], b_d[:, off : off + V]
                ).then_inc(s_dma_b, 16)
                if c >= 2:
                    sync.wait_ge(s_sq_a, c - 1)
                sync.dma_start(
                    tfa[k][:, 0:V], a_d[:, off : off + V]
                ).then_inc(s_dma_a, 16)

        @block.gpsimd
        def _(g):
            for c in range(NCH):
                k = c % 2
                V = VS[c]
                g.wait_ge(s_exp_b, c + 1)
                if c >= 1:
                    g.wait_ge(s_isb, c)      # t16b free
                g.memset(bch[k][:, B * V : S * V], 1.0)
                g.tensor_tensor(
                    t16b[:, 0 : 16 * V], bch[k][:, 0 : 16 * V],
                    bch[k][:, 16 * V : 32 * V], ALU.add,
                ).then_inc(s_l1b, 1)
                g.wait_ge(s_exp_a, c + 1)
                if c >= 1:
                    g.wait_ge(s_mul2, c)     # t16a free
                g.tensor_tensor(
                    t16a[:, 0 : 16 * V], ach[k][:, 0 : 16 * V],
                    ach[k][:, 16 * V : 32 * V], ALU.add,
                ).then_inc(s_l1a, 1)
            g.wait_ge(s_done, 1)
            g.dma_start(out_d[:, :], stats_sb[:, :]).then_inc(s_out, 16)
            g.wait_ge(s_out, 16)

        def l2plus_recip(v, t16, V, ALU):
            v.tensor_tensor(
                t16[:, 0 : 8 * V], t16[:, 0 : 8 * V], t16[:, 8 * V : 16 * V],
                ALU.add,
            )
            v.tensor_tensor(
                t16[:, 0 : 4 * V], t16[:, 0 : 4 * V], t16[:, 4 * V : 8 * V],
                ALU.add,
            )
            v.tensor_tensor(
                t16[:, 0 : 2 * V], t16[:, 0 : 2 * V], t16[:, 2 * V : 4 * V],
                ALU.add,
            )
            v.tensor_tensor(
                s1[:, 0:V], t16[:, 0:V], t16[:, V : 2 * V], ALU.add
            )
            v.tensor_scalar(
                h1[:, 0:V], s1[:, 0:V], float(_RC2), float(_RC1), ALU.mult,
                ALU.add,
            )
            v.tensor_tensor(y1[:, 0:V], s1[:, 0:V], h1[:, 0:V], ALU.mult)
            v.tensor_scalar(
                y1[:, 0:V], y1[:, 0:V], 1.0, float(_RC0), ALU.mult, ALU.add
            )

        @block.vector
        def _(v):
            from concourse import mybir
            ALU = mybir.AluOpType
            for c in range(NCH + 1):
                k = c % 2
                kp = (c - 1) % 2
                if c < NCH:
                    V = VS[c]
                    # head: b-side first
                    v.wait_ge(s_dma_b, 16 * (c + 1))
                    v.tensor_scalar(
                        tbb[k][:, 0:V], tfb[k][:, 0:V], float(_SQP), None,
                        ALU.mult,
                    )
                    if c >= 2:
                        v.wait_ge(s_pe, c - 1)
                    for j in range(B):
                        ts = v.tensor_scalar(
                            bch[k][:, j * V : (j + 1) * V], tbb[k][:, 0:V],
                            1.0, float(-_SQP * _BC[j]), ALU.mult, ALU.add,
                        )
                        if j == B - 1:
                            ts.then_inc(s_sub_b, 1)
                    v.wait_ge(s_dma_a, 16 * (c + 1))
                    v.tensor_scalar(
                        tba[k][:, 0:V], tfa[k][:, 0:V], float(_SQP), None,
                        ALU.mult,
                    )
                    for j in range(B):
                        v.tensor_scalar(
                            ach[k][:, j * V : (j + 1) * V], tba[k][:, 0:V],
                            1.0, float(-_SQP * _BC[j]), ALU.mult, ALU.add,
                        )
                    v.tensor_tensor(
                        ach[k][:, 0 : B * V], ach[k][:, 0 : B * V],
                        ach[k][:, 0 : B * V], ALU.mult,
                    ).then_inc(s_sq_a, 1)
                if c >= 1:
                    Vp = VS[c - 1]
                    # tail for chunk c-1: b-side (isb) first, then a-side
                    v.wait_ge(s_l1b, c)
                    l2plus_recip(v, t16b, Vp, ALU)
                    v.tensor_scalar(
                        ach[kp][:, B * Vp : S * Vp], y1[:, 0:Vp], 0.0, None,
                        ALU.add,
                    ).then_inc(s_isb, 1)
                    v.wait_ge(s_l1a, c)
                    l2plus_recip(v, t16a, Vp, ALU)
                    v.tensor_scalar(
                        yb[:, 0:Vp], y1[:, 0:Vp], 0.0, None, ALU.add
                    )
                    H = Vp // 2
                    a3 = ach[kp][:, 0 : B * Vp].rearrange(
                        "p (j v) -> p j v", v=Vp
                    )
                    y3 = (
                        yb[:, 0:Vp]
                        .rearrange("p (o v) -> p o v", o=1)
                        .broadcast_to([P, B, Vp])
                    )
                    v.tensor_tensor(
                        a3[:, :, 0:H], a3[:, :, 0:H], y3[:, :, 0:H],
                        ALU.mult,
                    ).then_inc(s_mul1, 1)
                    v.tensor_tensor(
                        a3[:, :, H:Vp], a3[:, :, H:Vp], y3[:, :, H:Vp],
                        ALU.mult,
                    ).then_inc(s_mul2, 1)
            v.wait_ge(s_pe, NCH)
            v.tensor_copy(stats_sb[:, :], acc[:, :]).then_inc(s_done, 1)

        @block.scalar
        def _(s):
            for c in range(NCH):
                k = c % 2
                V = VS[c]
                s.wait_ge(s_sub_b, c + 1)
                s.activation(
                    bch[k][:, 0 : B * V], bch[k][:, 0 : B * V], AF.Square
                )
                s.activation(
                    bch[k][:, 0 : B * V], bch[k][:, 0 : B * V], AF.Exp,
                    scale=-1.0,
                ).then_inc(s_exp_b, 1)
                s.wait_ge(s_sq_a, c + 1)
                s.activation(
                    ach[k][:, 0 : B * V], ach[k][:, 0 : B * V], AF.Exp,
                    scale=-1.0,
                ).then_inc(s_exp_a, 1)

        @block.tensor
        def _(t):
            for c in range(NCH):
                k = c % 2
                V = VS[c]
                H = V // 2
                t.wait_ge(s_l1b, c + 1)   # bch bins + ones ready
                t.wait_ge(s_isb, c + 1)
                t.wait_ge(s_mul1, c + 1)
                lv = ach[k][:, 0 : S * V].rearrange("p (j v) -> p v j", v=V)
                rv = bch[k][:, 0 : S * V].rearrange("p (j v) -> p v j", v=V)
                for vv in range(H):
                    mm = t.matmul(
                        acc[:, :],
                        lv[:, vv, :],
                        rv[:, vv, :],
                        start=(c == 0 and vv == 0),
                        stop=False,
                    )
                t.wait_ge(s_mul2, c + 1)
                for vv in range(H, V):
                    last = c == NCH - 1 and vv == V - 1
                    mm = t.matmul(
                        acc[:, :],
                        lv[:, vv, :],
                        rv[:, vv, :],
                        start=False,
                        stop=last,
                    )
                    if vv == V - 1:
                        mm.then_inc(s_pe, 1)

    return nc


def _get_nc():
    if "nc" not in _CACHE:
        _CACHE["nc"] = _build_nc()
    return _CACHE["nc"]


def run_device(a_flat, b_flat, trace=False):
    from concourse.bass_utils import run_bass_kernel_spmd

    nc = _get_nc()

    def shard(x, i):
        return np.ascontiguousarray(
            x[i * NVOX : (i + 1) * NVOX].reshape(P, COLS)
        )

    in_maps = []
    for i in range(NCORES):
        in_maps.append({"a": shard(a_flat, i), "b": shard(b_flat, i)})
    kw = {}
    if trace:
        kw.update(trace=True, trace_cores=[0])
    res = run_bass_kernel_spmd(nc, in_maps, list(range(NCORES)), **kw)
    stats = np.zeros((S, S), np.float64)
    for r in res.results:
        stats += np.asarray(r["stats"], np.float64)
    return stats, res


def finish(stats):
    n = float(NVOX_TOTAL)
    pab = stats[0:B, 0:B] / n
    pa = stats[0:B, B] / n
    pb = stats[B, 0:B] / n
    eps = 1.4e-45
    papb = np.outer(pa, pb) + eps
    mi = np.sum(pab * np.log(pab / papb + eps))
    return np.array([-mi], dtype=np.float32)


def kernel(actual, target):
    a = np.clip(np.asarray(actual, np.float32).reshape(-1), 0.0, 1.0)
    b = np.clip(np.asarray(target, np.float32).reshape(-1), 0.0, 1.0)
    stats, _ = run_device(a, b)
    return finish(stats)
